# revision 32
# baseline (speedup 1.0000x reference)
"""Trainium2 Bass kernel for nn_Attention_49366354100559.

Multi-head attention: B=2, T=2048, D=768, H=12, Dh=64.
Reference zeroes the upper triangle of scores (not -inf) before softmax,
so masked positions contribute exp(0)=1 to the softmax — the attention
matrix is dense in attn@v.

Sharding: 8 cores = 2 batches x 4 core-groups; each core computes 3 heads
of one batch and produces a partial [2048, 768] output (pre-W_o-bias);
host sums the 4 partials per batch and adds b_o.

Per-core device program (fp16 matmul dtype):
  1. Inputs arrive fp16.  All weights ride in one [128,4736] "wall" blob
     (one DMA); x rides in four [128,3072] strided DMAs (one per
     512-token group) — per-dma_start descriptor generation costs ~0.7us
     of serial sequencer time, so DMA count is minimized.
  2. x^T via PE transposes (fp16 stationary -> fast weight load), PSUM
     fp16, evacuated by DVE.
  3. q^T,k^T feature-major with W stationary; v token-major with x^T
     stationary, 4 token-tiles packed per PSUM [128,1024] tile (one wide
     DVE copy per head per group); v bias folded into finalization.
  4. Attention pipelined at PSUM-tile granularity, J-major over units
     (h0h1-pair quad J, then h2 quad J) so each O-projection group fires as
     soon as its three heads finalize instead of in a serial tail.
     exp on ACT straight out of PSUM; causal edge via
     affine_select(fill=1.0); fully-masked k-tiles via per-quad v
     column-sum suffixes; vaug ones column accumulates the softmax
     denominator for free.
  5. Finalize: reciprocal of denominator row, partition-broadcast,
     scale + per-partition v-bias -> attn_out^T.  y written fp16, 4
     token-tiles per DMA; host sums partials in fp32 and adds b_o.
"""

import sys

import numpy as np

if "/opt/trn_rl_repo" not in sys.path:
    sys.path.insert(0, "/opt/trn_rl_repo")

import concourse.mybir as mybir
from concourse import bacc
from concourse.tile import TileContext
from concourse.bass_utils import run_bass_kernel_spmd

F32 = mybir.dt.float32
F16 = mybir.dt.float16
AF = mybir.ActivationFunctionType
ALU = mybir.AluOpType

N_CORES = 8
T = 2048
D = 768
HPC = 3  # heads per core
DH = 64
NK = 16  # k-token tiles of 128
NG = 4  # q groups of 512
KT = 6  # contraction tiles for D=768

# wall blob column offsets
W_ONES = 0
W_QK = 512
W_V = W_QK + 6 * 384
W_O01 = W_V + 6 * 192
W_COLS = W_O01 + 768


def build_nc():
    nc = bacc.Bacc("TRN2", target_bir_lowering=False, debug=False,
                   num_devices=N_CORES)
    d = {}
    d["x"] = nc.dram_tensor("x", [T, D], F16, kind="ExternalInput").ap()
    d["wall"] = nc.dram_tensor("wall", [128, W_COLS], F16,
                               kind="ExternalInput").ap()
    d["wo2"] = nc.dram_tensor("wo2", [DH, D], F16,
                              kind="ExternalInput").ap()
    d["wbias"] = nc.dram_tensor("wbias", [128, 8], F32,
                                kind="ExternalInput").ap()
    d["ident"] = nc.dram_tensor("ident", [128, 128], F16,
                                kind="ExternalInput").ap()
    d["y"] = nc.dram_tensor("y", [T, D], F16, kind="ExternalOutput").ap()

    with TileContext(nc) as tc:
        _emit(nc, tc, d)
    nc.compile()
    return nc


def _emit(nc, tc, d):
    from contextlib import ExitStack

    with ExitStack() as ctx:
        wp = ctx.enter_context(tc.tile_pool(name="wp", bufs=1))
        main = ctx.enter_context(tc.tile_pool(name="main", bufs=1))

        # ---- weight/constant tiles ----
        wall = wp.tile([128, W_COLS], F16, tag="wall", name="wall")
        wo2 = wp.tile([DH, D], F16, tag="wo2", name="wo2")
        wbias = wp.tile([128, 8], F32, tag="wbias", name="wbias")
        ident = wp.tile([128, 128], F16, tag="ident", name="ident")

        ones = wall[:, W_ONES:W_ONES + 512]

        def wqk(k, c0, c1):
            return wall[:, W_QK + k * 384 + c0:W_QK + k * 384 + c1]

        def wv(k):
            return wall[:, W_V + k * 192:W_V + (k + 1) * 192]

        wo01 = wall[:, W_O01:W_O01 + 768]
        bqk = wbias[:, 0:3]
        bv = wbias[0:64, 3:6]

        # ---- persistent SBUF ----
        xT = [main.tile([128, T], F16, tag=f"xT{f}", name=f"xT{f}")
              for f in range(KT)]
        qkt = [main.tile([128, T], F16, tag=f"qkt{g}", name=f"qkt{g}")
               for g in range(3)]  # [q0|q1], [k0|k1], [q2|k2]
        alt2 = main.tile([128, T], F16, tag="alt2", name="alt2")
        vaug = [main.tile([128, NK * 65], F16, tag=f"vaug{h}",
                          name=f"vaug{h}") for h in range(HPC)]
        aout01 = main.tile([128, T], F16, tag="aout01", name="aout01")
        aout1 = main.tile([DH, T], F16, tag="aout1", name="aout1")
        aout2 = main.tile([DH, T], F16, tag="aout2", name="aout2")
        accs = [[main.tile([65, 512], F32, tag=f"acc{h}{g}",
                           name=f"acc{h}{g}") for g in range(NG)]
                for h in range(HPC)]
        vsum = [[main.tile([128, 65], F16, tag=f"vs{h}{g}",
                           name=f"vs{h}{g}") for g in range(3)]
                for h in range(HPC)]

        # ---- phase 0: DMAs (few, fat) ----
        nc.scalar.dma_start(ident[:], d["ident"])
        nc.scalar.dma_start(wall[:], d["wall"])
        nc.scalar.dma_start(wbias[:], d["wbias"])
        nc.scalar.dma_start(wo2[:], d["wo2"])

        for h in range(HPC):
            nc.vector.tensor_copy(
                vaug[h].rearrange("p (k c) -> p k c", c=65)[:, :, 64],
                ones[:, 0:1].broadcast_to([128, NK]))

        psA = ctx.enter_context(
            tc.tile_pool(name="psA", bufs=2, space="PSUM"))
        psB = ctx.enter_context(
            tc.tile_pool(name="psB", bufs=3, space="PSUM"))

        # ---- PE warmup: un-throttle the HAM clock gate during DMA wait ----
        wm = psA.tile([128, 512], F32, tag="qo", name="warm")
        wmout = main.tile([1, 128], F32, tag="wmout", name="wmout")
        for i in range(32):
            nc.tensor.matmul(wm[:, 0:128], ident[:], ident[:],
                             start=(i == 0), stop=(i == 31))
        nc.vector.tensor_copy(wmout[:], wm[0:1, 0:128])

        # ---- phase 1: x^T via PE transposes on fp16 x ----
        with tc.tile_pool(name="xp", bufs=2) as xp:
            for tq in range(4):
                x4 = xp.tile([128, 4 * D], F16, tag="x4", name=f"x4_{tq}")
                nc.sync.dma_start(
                    x4.rearrange("p (j c) -> p j c", c=D),
                    d["x"][tq * 512:(tq + 1) * 512, :].rearrange(
                        "(j p) c -> p j c", p=128))
                for f in range(KT):
                    ps = psB.tile([128, 512], F16, tag="sv",
                                  name=f"tp{tq}_{f}")
                    for j in range(4):
                        nc.tensor.transpose(
                            ps[:, j * 128:(j + 1) * 128],
                            x4[:, j * D + f * 128:j * D + (f + 1) * 128],
                            ident[:])
                    dst = xT[f][:, tq * 512:(tq + 1) * 512]
                    if f % 2 == 0:
                        nc.vector.tensor_copy(dst, ps[:])
                    else:
                        nc.scalar.copy(dst, ps[:])  # ACT idle in phase 1

        # ---- phase 2a: q/k projections for heads 0,1 (earliest) ----
        for n in range(NG):
            for g in range(2):
                ps = psA.tile([128, 512], F32, tag="qo", name=f"qk{g}_{n}")
                for k in range(KT):
                    nc.tensor.matmul(
                        ps[:], wqk(k, g * 128, (g + 1) * 128),
                        xT[k][:, n * 512:(n + 1) * 512],
                        start=(k == 0), stop=(k == KT - 1))
                nc.scalar.activation(
                    qkt[g][:, n * 512:(n + 1) * 512], ps[:],
                    AF.Identity, bias=bqk[:, g:g + 1])

        # ---- fillers: h2 q/k proj, alt2, v-projection, vsum ----
        def make_g2(n):
            def fil():
                ps = psA.tile([128, 512], F32, tag="qo", name=f"qk2_{n}")
                for k in range(KT):
                    nc.tensor.matmul(
                        ps[:], wqk(k, 256, 384),
                        xT[k][:, n * 512:(n + 1) * 512],
                        start=(k == 0), stop=(k == KT - 1))
                nc.vector.tensor_scalar_add(
                    qkt[2][:, n * 512:(n + 1) * 512], ps[:], bqk[:, 2:3])
            return fil

        def make_alt2():
            def fil():
                nc.sync.dma_start(alt2[0:64, :], qkt[2][64:128, :])
                nc.sync.dma_start(alt2[64:128, :], qkt[2][0:64, :])
            return fil

        def make_v4(n):
            def fil():
                ps4 = psB.tile([128, 1024], F32, tag="sv", name=f"v4_{n}")
                for tl in range(4):
                    tt = 4 * n + tl
                    for k in range(KT):
                        nc.tensor.matmul(
                            ps4[:, tl * 256:tl * 256 + 192],
                            xT[k][:, tt * 128:(tt + 1) * 128], wv(k),
                            start=(k == 0), stop=(k == KT - 1))
                ps3 = ps4.rearrange("p (t c) -> p t c", c=256)
                for h in range(HPC):
                    nc.vector.tensor_copy(
                        vaug[h].rearrange("p (k c) -> p k c", c=65)
                        [:, 4 * n:4 * n + 4, 0:64],
                        ps3[:, :, 64 * h:64 * h + 64])
            return fil

        def make_vsum():
            def fil():
                for h in range(HPC):
                    va3 = vaug[h].rearrange("p (k c) -> p c k", c=65)
                    for g in range(3):
                        v32 = main.tile([128, 65], F32, tag="v32",
                                        name=f"v32_{h}{g}", bufs=2)
                        nc.vector.tensor_reduce(
                            v32[:], va3[:, :, 4 * (g + 1):NK],
                            axis=mybir.AxisListType.X, op=ALU.add)
                        nc.vector.tensor_copy(vsum[h][g][:], v32[:])
            return fil

        fillers = [make_g2(n) for n in range(NG)]
        fillers.append(make_alt2())
        fillers += [make_v4(n) for n in range(NG)]
        fillers.append(make_vsum())

        # ---- phase 3+4: attention pipeline + O-projection ----
        pair01 = [
            (qkt[0][0:64, :], qkt[1][0:64, :]),      # h0: base 0
            (qkt[0][64:128, :], qkt[1][64:128, :]),  # h1: base 64
        ]
        h2qk = [
            (qkt[2][0:64, :], alt2[0:64, :]),        # h2 even ki: base 0
            (alt2[64:128, :], qkt[2][64:128, :]),    # h2 odd ki: base 64
        ]
        fill1 = nc.gpsimd.to_reg(1.0)

        with tc.tile_pool(name="ep", bufs=14) as ep, \
             tc.tile_pool(name="fin", bufs=2) as fin, \
             tc.tile_pool(name="outp", bufs=2) as outp:

            erows = {}

            def new_erow(h, ki):
                e = ep.tile([128, T], F16, tag="e", name=f"e{h}_{ki}")
                erows[(h, ki)] = e
                return e

            def score_P(J, P, items):
                # items: (h, ki, qT, kT) sharing this time slot; adjacent
                # emission per column block (row-group pairing).
                pss = {}
                for h, ki, qT, kT in items:
                    pss[(h, ki)] = psB.tile([128, 1024], F32, tag="sv",
                                            name=f"s{h}_{ki}_{P}")
                for nb in range(2):
                    s0 = 1024 * P + 512 * nb
                    for h, ki, qT, kT in items:
                        lo = 128 * ki
                        if s0 + 512 <= lo:
                            continue
                        a0 = max(s0, lo)
                        nc.tensor.matmul(
                            pss[(h, ki)][:, a0 - 1024 * P:512 * (nb + 1)],
                            kT[:, lo:lo + 128], qT[:, a0:s0 + 512])
                for h, ki, qT, kT in items:
                    lo = 128 * ki
                    clo = max(lo, 1024 * P)
                    nc.scalar.activation(
                        erows[(h, ki)][:, clo:1024 * (P + 1)],
                        pss[(h, ki)][:, clo - 1024 * P:1024], AF.Exp,
                        scale=0.125)

            def edge_AS(J, items):
                for h, ki, qT, kT in items:
                    lo = 128 * ki
                    w = lo + 128 - 512 * J
                    nc.gpsimd.affine_select(
                        erows[(h, ki)][:, 512 * J:lo + 128],
                        erows[(h, ki)][:, 512 * J:lo + 128],
                        pattern=[[1, w]], compare_op=ALU.is_ge,
                        fill=fill1, base=512 * J - lo,
                        channel_multiplier=-1)

            def unit_score_steps(u):
                kind, J = u
                steps = []
                if kind == "p":
                    kisets = [[4 * J + j] for j in range(4)]
                else:
                    kisets = [[4 * J + 2 * t, 4 * J + 2 * t + 1]
                              for t in range(2)]
                for kis in kisets:
                    if kind == "p":
                        items = [(h, kis[0], pair01[h][0], pair01[h][1])
                                 for h in range(2)]
                    else:
                        items = [(2, ki, h2qk[ki % 2][0], h2qk[ki % 2][1])
                                 for ki in kis]
                    for h, ki, qT, kT in items:
                        new_erow(h, ki)
                    p_lo = min(128 * ki // 1024 for ki in kis)
                    first = True
                    for P in range(p_lo, 2):
                        def step(P=P, items=items, first=first, J=J):
                            score_P(J, P, items)
                            if first:
                                edge_AS(J, items)
                        steps.append(step)
                        first = False
                return steps

            def unit_attnv_groups(u):
                kind, J = u
                heads = [0, 1] if kind == "p" else [2]
                groups = []
                # non-diagonal groups first: the diagonal (g==J) needs
                # vsum + feeds finalize, so it gets max slack
                for g in list(range(J + 1, NG)) + [J]:
                    for h in heads:

                        def grp(g=g, h=h, J=J):
                            po = psA.tile([65, 512], F32, tag="qo",
                                          name=f"o{h}{J}{g}")
                            has_virtual = (J == g and g < 3)
                            for j in range(4):
                                ki = 4 * J + j
                                nc.tensor.matmul(
                                    po[:],
                                    vaug[h][:, ki * 65:ki * 65 + 65],
                                    erows[(h, ki)][:, 512 * g:512 * (g + 1)],
                                    start=(j == 0),
                                    stop=(j == 3 and not has_virtual))
                            if has_virtual:
                                nc.tensor.matmul(po[:], vsum[h][g][:],
                                                 ones[:, 0:512],
                                                 start=False, stop=True)
                            if J == 0:
                                nc.vector.tensor_copy(accs[h][g][:], po[:])
                            else:
                                nc.vector.tensor_add(accs[h][g][:], po[:],
                                                     accs[h][g][:])
                        groups.append(grp)
                # finalize + O-projection as separate pending items so the
                # next unit's score MMs interleave into their latency
                for h in heads:
                    groups.append(lambda h=h, J=J: finalize(h, J))
                if kind == "2":
                    groups.append(lambda J=J: oproj_group(J))
                return groups

            def finalize(h, g):
                blk = slice(512 * g, 512 * (g + 1))
                den = fin.tile([1, 512], F32, tag="den", name=f"den{h}{g}")
                scr = fin.tile([1, 512], F32, tag="scr", name=f"scr{h}{g}")
                sch = fin.tile([1, 512], F16, tag="sch", name=f"sch{h}{g}")
                nc.vector.tensor_copy(den[:], accs[h][g][64:65, :])
                nc.vector.reciprocal_approx_fast(scr[:], den[:])
                nc.vector.tensor_copy(sch[:], scr[:])
                # broadcast 1/den across 64 partitions with a K=1 matmul
                rb = psA.tile([DH, 512], F32, tag="qo", name=f"rb{h}{g}")
                nc.tensor.matmul(rb[:], ones[0:1, 0:DH], sch[:],
                                 start=True, stop=True)
                if h == 0:
                    dst = aout01[0:64, blk]
                elif h == 1:
                    dst = aout1[:, blk]
                else:
                    dst = aout2[:, blk]
                nc.vector.tensor_mul(dst, accs[h][g][0:64, :], rb[:])
                nc.vector.tensor_scalar_add(dst, dst, bv[:, h:h + 1])
                if h == 1:
                    # stack h1 under h0 (partition-shift DMA)
                    nc.sync.dma_start(aout01[64:128, blk], aout1[:, blk])

            def oproj_group(tg):
                ot4 = outp.tile([128, 4 * D], F16, tag="ot", name=f"ot{tg}")
                for tl in range(4):
                    tt = 4 * tg + tl
                    # both 512/256 column blocks in one 2-bank tile ->
                    # one wide evacuation copy per token tile
                    ps = psB.tile([128, 1024], F32, tag="sv",
                                  name=f"op{tt}")
                    for (n0, w) in ((0, 512), (512, 256)):
                        nc.tensor.matmul(
                            ps[:, n0:n0 + w],
                            aout01[:, tt * 128:(tt + 1) * 128],
                            wo01[:, n0:n0 + w], start=True, stop=False)
                        nc.tensor.matmul(
                            ps[:, n0:n0 + w],
                            aout2[:, tt * 128:(tt + 1) * 128],
                            wo2[:, n0:n0 + w], start=False, stop=True)
                    if tg >= 2:
                        # late groups: exp is done, ACT is idle, DVE is the
                        # tail bottleneck — evacuate on ACT instead
                        nc.scalar.copy(ot4[:, tl * D:tl * D + D],
                                       ps[:, 0:D])
                    else:
                        nc.vector.tensor_copy(
                            ot4[:, tl * D:tl * D + D], ps[:, 0:D])
                nc.sync.dma_start(
                    d["y"][tg * 512:(tg + 1) * 512, :].rearrange(
                        "(j p) c -> p j c", p=128),
                    ot4.rearrange("p (j c) -> p j c", c=D))

            units = []
            for J in range(NG):
                units.append(("p", J))
                units.append(("2", J))
            pending = fillers
            for u in units:
                steps = unit_score_steps(u)
                per = (len(pending) + len(steps) - 1) // max(len(steps), 1)
                gi = 0
                for st in steps:
                    st()
                    for _ in range(per):
                        if gi < len(pending):
                            pending[gi]()
                            gi += 1
                while gi < len(pending):
                    pending[gi]()
                    gi += 1
                pending = unit_attnv_groups(u)
            for grp in pending:
                grp()


_NC_CACHE = None


def _get_nc():
    global _NC_CACHE
    if _NC_CACHE is None:
        _NC_CACHE = build_nc()
    return _NC_CACHE


def _make_in_maps(residual_stream, W_q, b_q, W_k, b_k, W_v, b_v, W_o, b_o):
    in_maps = []
    for c in range(N_CORES):
        b = c // 4
        hs = [3 * (c % 4) + i for i in range(HPC)]
        cs = [slice(64 * h, 64 * h + 64) for h in hs]
        wqk = np.concatenate(
            [W_q[:, cs[0]], W_q[:, cs[1]], W_k[:, cs[0]], W_k[:, cs[1]],
             W_q[:, cs[2]], W_k[:, cs[2]]], axis=1).astype(np.float16)
        wv = np.concatenate(
            [W_v[:, s] for s in cs], axis=1).astype(np.float16)
        wo01 = W_o[64 * hs[0]:64 * hs[0] + 128, :].astype(np.float16)
        wall = np.concatenate(
            [np.ones((128, 512), dtype=np.float16)]
            + [wqk[k * 128:(k + 1) * 128, :] for k in range(KT)]
            + [wv[k * 128:(k + 1) * 128, :] for k in range(KT)]
            + [wo01], axis=1)
        assert wall.shape == (128, W_COLS)
        bqk = np.concatenate(
            [b_q[cs[0]], b_q[cs[1]], b_k[cs[0]], b_k[cs[1]],
             b_q[cs[2]], b_k[cs[2]]]).astype(np.float32)
        wbias = np.zeros((128, 8), dtype=np.float32)
        wbias[:, 0:3] = bqk.reshape(3, 128).T
        wbias[0:64, 3:6] = np.stack([b_v[s] for s in cs], axis=1)
        m = {
            "x": np.ascontiguousarray(residual_stream[b]).astype(np.float16),
            "wall": np.ascontiguousarray(wall),
            "wbias": wbias,
            "wo2": np.ascontiguousarray(
                W_o[64 * hs[2]:64 * hs[2] + 64, :]).astype(np.float16),
            "ident": np.eye(128, dtype=np.float16),
        }
        in_maps.append(m)
    return in_maps


def kernel(residual_stream, W_q, b_q, W_k, b_k, W_v, b_v, W_o, b_o,
           _trace=False):
    residual_stream = np.asarray(residual_stream, dtype=np.float32)
    args = [np.asarray(a, dtype=np.float32)
            for a in (W_q, b_q, W_k, b_k, W_v, b_v, W_o, b_o)]
    W_q, b_q, W_k, b_k, W_v, b_v, W_o, b_o = args
    nc = _get_nc()
    in_maps = _make_in_maps(residual_stream, W_q, b_q, W_k, b_k, W_v, b_v,
                            W_o, b_o)
    res = run_bass_kernel_spmd(nc, in_maps, core_ids=list(range(N_CORES)),
                               trace=_trace)
    B = residual_stream.shape[0]
    out = np.zeros((B, T, D), dtype=np.float32)
    for c in range(N_CORES):
        out[c // 4] += res.results[c]["y"].astype(np.float32)
    out += b_o[None, None, :]
    if _trace:
        kernel._last_result = res
    return out
